# revision 4
# baseline (speedup 1.0000x reference)
"""Trainium2 Bass kernel for nn_LocalNetwork (avgpool3d -> 3x LocallyConnected1D -> upsample3d).

Sharding: pure data parallelism — batch 256 split as 32 per core across 8 cores.

Per-core design (32 batches = 4 load groups of 8; conv pairs of 2 groups):
  partition p = (bl, dslice)  [8 x 15 = 120 partitions], free = (h, w).

v2 changes vs v1 (167.5us):
  - Loads split into TWO free-half tiles per group (16KB descriptors; the
    8KB descriptors of v1 capped the per-SDMA-engine rate at ~14 B/ns vs
    ~23.5 at 16KB; 15 engines x 23.5 ~= 350 GB/s ~ HBM cap).
  - Loads are spread across BOTH HWDGE queues (sync q1: g0/g2, scalar
    q10: g1/g3) and stores are interleaved behind them in readiness
    order, so read+write traffic overlaps from ~35us instead of
    serializing (v1: all 16.8MB of loads completed before any store).
  - Weight replication to partitions 64:104 is a second 1MB DMA instead
    of 12 identity matmuls + scalar copies (frees ~25us of PE time and
    the scalar engine, which now only triggers q10 DMAs).
  - Upsample h-expands + one w-expand half moved to GpSimd (1-input ops
    run at ~line rate there); DVE keeps pools + conv chains + the other
    w-expand half, cutting DVE busy from ~90us to ~65us.
  - tile_wait_until removed; emission order provides the DVE schedule:
    pool g0, g1 -> conv pair0 -> pool g2, g3 -> conv pair1.
"""

import numpy as np

import concourse.bass as bass
import concourse.mybir as mybir
from concourse import bacc
from concourse.bass_utils import run_bass_kernel_spmd
from concourse.tile import TileContext

F32 = mybir.dt.float32
ADD = mybir.AluOpType.add
MULT = mybir.AluOpType.mult

N_CORES = 8
B = 256
B_CORE = 32          # batches per core
G = 4                # load groups per core
B_GRP = 8            # batches per group
CORE_ELEMS = B_CORE * 15 * 64 * 128  # 3,932,160
BSTRIDE = 15 * 64 * 128              # 122,880
SLICE = 64 * 128                     # 8192 elems = one (h,w) plane = 32KB
HALF = SLICE // 2                    # 4096 elems = 16KB descriptor runs


def _pack_consts(w_depth, b_depth, w_lon, b_lon, w_lat, b_lat):
    """Returns (mm [120,128] f32, wts [40,6144] f32).

    mm: three matmul lhsT tiles [120,40] (cols 0:40 dn / 40:80 mid / 80:120 up)
        out[q=(bl,dp), f] = sum_p lhsT[p=(bl,dsl), q] * P2[p, f]
        coefficient 1/48 folds the avg-pool mean.
    wts: 12 x [40,512] conv weight/bias tiles, p=(bl,dp), f=(ho,wo).
    """
    mm = np.zeros((120, 128), np.float32)
    for bl in range(8):
        for dsl in range(15):
            p = bl * 15 + dsl
            grp = dsl // 3
            for col0, dp in ((0, grp + 1), (40, grp), (80, grp - 1)):
                if 0 <= dp <= 4:
                    mm[p, col0 + bl * 5 + dp] = 1.0 / 48.0

    dp = np.arange(5)[:, None, None]
    ho = np.arange(16)[None, :, None]
    wo = np.arange(32)[None, None, :]
    ld = wo * 112 + ho * 7 + (dp + 1)     # depth seq index (5,16,32)
    ll = dp * 544 + ho * 34 + (wo + 1)    # lon
    lt = dp * 576 + wo * 18 + (ho + 1)    # lat

    def tile(vec, idx):
        t = np.broadcast_to(np.asarray(vec)[idx][None], (8, 5, 16, 32))
        return t.reshape(40, 512)

    cols = []
    for j in range(3):
        cols.append(tile(np.asarray(w_depth)[:, j], ld))
    cols.append(tile(b_depth, ld))
    for j in range(3):
        cols.append(tile(np.asarray(w_lon)[:, j], ll))
    cols.append(tile(b_lon, ll))
    for j in range(3):
        cols.append(tile(np.asarray(w_lat)[:, j], lt))
    cols.append(tile(b_lat, lt))
    wts = np.concatenate(cols, axis=1)
    return mm, np.ascontiguousarray(wts, dtype=np.float32)


def build_nc(reps: int = 1) -> bass.Bass:
    nc = bacc.Bacc("TRN2", target_bir_lowering=False, debug=False)
    x = nc.dram_tensor("x", [CORE_ELEMS], F32, kind="ExternalInput")
    mmc = nc.dram_tensor("mm", [120, 128], F32, kind="ExternalInput")
    wtc = nc.dram_tensor("wts", [40, 6144], F32, kind="ExternalInput")
    y = nc.dram_tensor("y", [CORE_ELEMS], F32, kind="ExternalOutput")

    with TileContext(nc) as tc:
        with (
            tc.tile_pool(name="cpool", bufs=1) as cpool,
            tc.tile_pool(name="inp", bufs=2) as inp,
            tc.tile_pool(name="outp", bufs=2) as outp,
            tc.tile_pool(name="work", bufs=1) as work,
            tc.tile_pool(name="p2p", bufs=2) as p2p,
            tc.tile_pool(name="psum", bufs=2, space="PSUM") as psum,
        ):
            MM = cpool.tile([120, 128], F32)
            WT = cpool.tile([104, 6144], F32)

            w = lambda i: WT[:, i * 512:(i + 1) * 512]
            wd0, wd1, wd2, bd = (w(i) for i in range(4))
            vl0, vl1, vl2, blon = (w(i) for i in range(4, 8))
            ul0, ul1, ul2, blat = (w(i) for i in range(8, 12))

            state = {}

            def qeng(g):
                # loads for g0/g2 + their pair's half-b stores on sync q;
                # g1/g3 + half-a stores on scalar q
                return nc.sync if (g % 2 == 0) else nc.scalar

            def load_half(g, c):
                # one [120, 4096] free-half: 120 x 16KB contiguous runs
                off = (g % G) * B_GRP * BSTRIDE + c * HALF
                X = inp.tile([120, HALF], F32, tag=f"x{c}")
                qeng(g).dma_start(
                    X[:],
                    bass.AP(x, off, [[BSTRIDE, 8], [SLICE, 15], [1, HALF]]))
                state[(g, c)] = X

            def load_consts_head():
                # depth-conv weights first (needed earliest), on both queues
                nc.sync.dma_start(MM[:], mmc[:])
                nc.sync.dma_start(WT[0:40, 0:2048], wtc[:, 0:2048])
                nc.scalar.dma_start(WT[64:104, 0:2048], wtc[:, 0:2048])

            def load_consts_tail():
                nc.sync.dma_start(WT[0:40, 2048:6144], wtc[:, 2048:6144])
                nc.scalar.dma_start(WT[64:104, 2048:6144], wtc[:, 2048:6144])

            def pool_half(g, c):
                # h,w avg-pool (sum) for free-half c -> P2[:, c*256:(c+1)*256]
                X = state.pop((g, c))
                if c == 0:
                    state[("P2", g)] = p2p.tile([120, 512], F32, tag="p2",
                                                name="P2")
                P2 = state[("P2", g)]
                nc.vector.tensor_reduce(
                    P2[:, c * 256:(c + 1) * 256]
                        .rearrange("p (ho wo) -> p ho wo", ho=8),
                    X[:].rearrange("p (ho hs wo ws) -> p ho wo hs ws",
                                   ho=8, hs=4, wo=32, ws=4),
                    mybir.AxisListType.XY, ADD)

            def mm_half(g, c):
                # depth pool (/48) + conv taps for free-half c
                k, half = divmod(g, 2)
                if half == 0 and c == 0:
                    Sdn = psum.tile([104, 512], F32)
                    S0 = psum.tile([104, 512], F32)
                    Sup = psum.tile([104, 512], F32)
                    state[("S", k)] = (Sdn, S0, Sup)
                Sdn, S0, Sup = state[("S", k)]
                P2 = state[("P2", g)]
                if c == 1:
                    state.pop(("P2", g))
                sl = slice(64 * half, 64 * half + 40)
                lo, hi = c * 256, (c + 1) * 256
                nc.tensor.matmul(Sdn[sl, lo:hi], MM[:, 0:40], P2[:, lo:hi],
                                 start=True, stop=True)
                nc.tensor.matmul(S0[sl, lo:hi], MM[:, 40:80], P2[:, lo:hi],
                                 start=True, stop=True)
                nc.tensor.matmul(Sup[sl, lo:hi], MM[:, 80:120], P2[:, lo:hi],
                                 start=True, stop=True)

            def conv_store_pair(k):
                ga, gb = 2 * k, 2 * k + 1
                Sdn, S0, Sup = state.pop(("S", k))
                # depth conv: 3 independent mults, then the add chain
                m = work.tile([104, 512], F32)
                m2 = work.tile([104, 512], F32)
                m3 = work.tile([104, 512], F32)
                nc.vector.tensor_tensor(m[:], wd0, Sdn[:], MULT)
                nc.vector.tensor_tensor(m2[:], wd1, S0[:], MULT)
                nc.vector.tensor_tensor(m3[:], wd2, Sup[:], MULT)
                nc.vector.tensor_tensor(m3[:], m3[:], bd, ADD)
                nc.vector.tensor_tensor(m[:], m[:], m2[:], ADD)
                nc.vector.tensor_tensor(m[:], m[:], m3[:], ADD)
                # relu into lon-padded tile Ydp[p, ho*34 + (wo+1)]
                Ydp = work.tile([104, 544], F32)
                Ydpv = Ydp[:].rearrange("p (ho wp) -> p ho wp", ho=16, wp=34)
                nc.gpsimd.memset(Ydpv[:, :, 0], 0)
                nc.gpsimd.memset(Ydpv[:, :, 33], 0)
                nc.vector.tensor_scalar_max(
                    Ydpv[:, :, 1:33],
                    m[:].rearrange("p (ho wo) -> p ho wo", ho=16), 0.0)

                # lon conv (along wo, free axis)
                m3v = m3[:].rearrange("p (ho wo) -> p ho wo", ho=16)
                mv = m[:].rearrange("p (ho wo) -> p ho wo", ho=16)
                m2v = m2[:].rearrange("p (ho wo) -> p ho wo", ho=16)
                w3 = lambda t: t.rearrange("p (ho wo) -> p ho wo", ho=16)
                nc.vector.tensor_tensor(mv, w3(vl0), Ydpv[:, :, 0:32], MULT)
                nc.vector.tensor_tensor(m2v, w3(vl1), Ydpv[:, :, 1:33], MULT)
                nc.vector.tensor_tensor(m3v, w3(vl2), Ydpv[:, :, 2:34], MULT)
                nc.vector.tensor_tensor(m3v, m3v, w3(blon), ADD)
                nc.vector.tensor_tensor(mv, mv, m2v, ADD)
                nc.vector.tensor_tensor(mv, mv, m3v, ADD)
                # relu into lat-padded tile Ylp[p, (ho+1)*32 + wo]
                Ylp = work.tile([104, 576], F32)
                nc.gpsimd.memset(Ylp[:, 0:32], 0)
                nc.gpsimd.memset(Ylp[:, 544:576], 0)
                nc.vector.tensor_scalar_max(Ylp[:, 32:544], m[:], 0.0)

                # lat conv (along ho, free axis; contiguous slices)
                nc.vector.tensor_tensor(m[:], ul0, Ylp[:, 0:512], MULT)
                nc.vector.tensor_tensor(m2[:], ul1, Ylp[:, 32:544], MULT)
                nc.vector.tensor_tensor(m3[:], ul2, Ylp[:, 64:576], MULT)
                nc.vector.tensor_tensor(m3[:], m3[:], blat, ADD)
                nc.vector.tensor_tensor(m[:], m[:], m2[:], ADD)
                nc.vector.tensor_tensor(m[:], m[:], m3[:], ADD)

                # upsample: relu + h-expand on gpsimd, then per-c w-expand
                # (c0 gpsimd / c1 DVE, so both U halves finish ~together)
                A = work.tile([104, 2048], F32)  # (ho, hs, wo)
                mv = m[:].rearrange("p (ho wo) -> p ho wo", ho=16)
                for c in range(2):
                    Av = A[:, c * 1024:(c + 1) * 1024].rearrange(
                        "p (ho hs wo) -> p ho hs wo", ho=8, hs=4)
                    mb = mv[:, c * 8:(c + 1) * 8].unsqueeze(2) \
                        .broadcast_to([104, 8, 4, 32])
                    nc.gpsimd.tensor_scalar_max(Av, mb, 0.0)
                for c in range(2):
                    U = outp.tile([104, HALF], F32, tag=f"u{c}")
                    Uw = U[:].rearrange("p (h wo ws) -> p h wo ws", h=32, ws=4)
                    Ab = A[:, c * 1024:(c + 1) * 1024].rearrange(
                        "p (h wo) -> p h wo", h=32) \
                        .unsqueeze(3).broadcast_to([104, 32, 32, 4])
                    eng = nc.gpsimd if c == 0 else nc.vector
                    eng.tensor_scalar_add(Uw, Ab, 0.0)
                    # stores: half-a (0:40) on scalar q, half-b (64:104)
                    # on sync q; x3 depth replication = 3 DMAs per half
                    for half, g in enumerate((ga, gb)):
                        off = (g % G) * B_GRP * BSTRIDE + c * HALF
                        for di in range(3):
                            eng2 = nc.scalar if half == 0 else nc.sync
                            eng2.dma_start(
                                bass.AP(y, off + di * SLICE,
                                        [[BSTRIDE, 8], [3 * SLICE, 5],
                                         [1, HALF]]),
                                U[64 * half:64 * half + 40, :])

            # software-pipelined emission; emission order = priority order
            for r in range(reps):
                b = r * G
                load_half(b + 0, 0)
                load_half(b + 1, 0)
                if r == 0:
                    load_consts_head()
                load_half(b + 0, 1)
                load_half(b + 1, 1)
                if r == 0:
                    load_consts_tail()
                pool_half(b + 0, 0)
                mm_half(b + 0, 0)
                load_half(b + 2, 0)
                load_half(b + 3, 0)
                pool_half(b + 1, 0)
                mm_half(b + 1, 0)
                load_half(b + 2, 1)
                load_half(b + 3, 1)
                pool_half(b + 0, 1)
                mm_half(b + 0, 1)
                pool_half(b + 1, 1)
                mm_half(b + 1, 1)
                conv_store_pair(b // 2 + 0)
                pool_half(b + 2, 0)
                mm_half(b + 2, 0)
                pool_half(b + 3, 0)
                mm_half(b + 3, 0)
                pool_half(b + 2, 1)
                mm_half(b + 2, 1)
                pool_half(b + 3, 1)
                mm_half(b + 3, 1)
                conv_store_pair(b // 2 + 1)

    nc.compile()
    return nc


_NC_CACHE = {}


def _get_nc(reps: int = 1):
    if reps not in _NC_CACHE:
        _NC_CACHE[reps] = build_nc(reps)
    return _NC_CACHE[reps]


def kernel(x, w_depth, b_depth, w_lon, b_lon, w_lat, b_lat, reps: int = 1,
           **run_kwargs):
    mm, wts = _pack_consts(w_depth, b_depth, w_lon, b_lon, w_lat, b_lat)
    xf = np.ascontiguousarray(np.asarray(x), dtype=np.float32).reshape(N_CORES, CORE_ELEMS)
    in_maps = [{"x": xf[c], "mm": mm, "wts": wts} for c in range(N_CORES)]
    nc = _get_nc(reps)
    res = run_bass_kernel_spmd(nc, in_maps, core_ids=list(range(N_CORES)), **run_kwargs)
    out = np.stack([r["y"] for r in res.results], axis=0)
    out = out.reshape(B, 15, 64, 128, 1)
    if run_kwargs:
        kernel.last_results = res
    return out


# revision 7
# speedup vs baseline: 1.7693x; 1.7693x over previous
"""Trainium2 Bass kernel for nn_LocalNetwork (avgpool3d -> 3x LocallyConnected1D -> upsample3d).

Sharding: pure data parallelism — batch 256 split as 32 per core across 8 cores.

Per-core design (32 batches = 4 load groups of 8; conv pairs of 2 groups):
  partition p = (bl, dslice)  [8 x 15 = 120 partitions], free = (h, w).

v2 changes vs v1 (167.5us):
  - Loads split into TWO free-half tiles per group (16KB descriptors; the
    8KB descriptors of v1 capped the per-SDMA-engine rate at ~14 B/ns vs
    ~23.5 at 16KB; 15 engines x 23.5 ~= 350 GB/s ~ HBM cap).
  - Loads are spread across BOTH HWDGE queues (sync q1: g0/g2, scalar
    q10: g1/g3) and stores are interleaved behind them in readiness
    order, so read+write traffic overlaps from ~35us instead of
    serializing (v1: all 16.8MB of loads completed before any store).
  - Weight replication to partitions 64:104 is a second 1MB DMA instead
    of 12 identity matmuls + scalar copies (frees ~25us of PE time and
    the scalar engine, which now only triggers q10 DMAs).
  - Upsample h-expands + one w-expand half moved to GpSimd (1-input ops
    run at ~line rate there); DVE keeps pools + conv chains + the other
    w-expand half, cutting DVE busy from ~90us to ~65us.
  - tile_wait_until removed; emission order provides the DVE schedule:
    pool g0, g1 -> conv pair0 -> pool g2, g3 -> conv pair1.
"""

import numpy as np

import concourse.bass as bass
import concourse.mybir as mybir
from concourse import bacc
from concourse.bass_utils import run_bass_kernel_spmd
from concourse.tile import TileContext

F32 = mybir.dt.float32
ADD = mybir.AluOpType.add
MULT = mybir.AluOpType.mult

N_CORES = 8
B = 256
B_CORE = 32          # batches per core
G = 4                # load groups per core
B_GRP = 8            # batches per group
CORE_ELEMS = B_CORE * 15 * 64 * 128  # 3,932,160
BSTRIDE = 15 * 64 * 128              # 122,880
SLICE = 64 * 128                     # 8192 elems = one (h,w) plane = 32KB
HALF = SLICE // 2                    # 4096 elems = 16KB descriptor runs


def _pack_consts(w_depth, b_depth, w_lon, b_lon, w_lat, b_lat):
    """Returns (mm [120,128] f32, wts [40,6144] f32).

    mm: three matmul lhsT tiles [120,40] (cols 0:40 dn / 40:80 mid / 80:120 up)
        out[q=(bl,dp), f] = sum_p lhsT[p=(bl,dsl), q] * P2[p, f]
        coefficient 1/48 folds the avg-pool mean.
    wts: 12 x [40,512] conv weight/bias tiles, p=(bl,dp), f=(ho,wo).
    """
    mm = np.zeros((120, 128), np.float32)
    for bl in range(8):
        for dsl in range(15):
            p = bl * 15 + dsl
            grp = dsl // 3
            for col0, dp in ((0, grp + 1), (40, grp), (80, grp - 1)):
                if 0 <= dp <= 4:
                    mm[p, col0 + bl * 5 + dp] = 1.0 / 48.0

    dp = np.arange(5)[:, None, None]
    ho = np.arange(16)[None, :, None]
    wo = np.arange(32)[None, None, :]
    ld = wo * 112 + ho * 7 + (dp + 1)     # depth seq index (5,16,32)
    ll = dp * 544 + ho * 34 + (wo + 1)    # lon
    lt = dp * 576 + wo * 18 + (ho + 1)    # lat

    def tile(vec, idx):
        t = np.broadcast_to(np.asarray(vec)[idx][None], (8, 5, 16, 32))
        return t.reshape(40, 512)

    cols = []
    for j in range(3):
        cols.append(tile(np.asarray(w_depth)[:, j], ld))
    cols.append(tile(b_depth, ld))
    for j in range(3):
        cols.append(tile(np.asarray(w_lon)[:, j], ll))
    cols.append(tile(b_lon, ll))
    for j in range(3):
        cols.append(tile(np.asarray(w_lat)[:, j], lt))
    cols.append(tile(b_lat, lt))
    wts = np.concatenate(cols, axis=1)
    return mm, np.ascontiguousarray(wts, dtype=np.float32)


def build_nc(reps: int = 1) -> bass.Bass:
    nc = bacc.Bacc("TRN2", target_bir_lowering=False, debug=False)
    x = nc.dram_tensor("x", [CORE_ELEMS], F32, kind="ExternalInput")
    mmc = nc.dram_tensor("mm", [120, 128], F32, kind="ExternalInput")
    wtc = nc.dram_tensor("wts", [40, 6144], F32, kind="ExternalInput")
    y = nc.dram_tensor("y", [CORE_ELEMS], F32, kind="ExternalOutput")

    with TileContext(nc) as tc:
        with (
            tc.tile_pool(name="cpool", bufs=1) as cpool,
            tc.tile_pool(name="inp", bufs=2) as inp,
            tc.tile_pool(name="outp", bufs=2) as outp,
            tc.tile_pool(name="work", bufs=1) as work,
            tc.tile_pool(name="p2p", bufs=2) as p2p,
            tc.tile_pool(name="psum", bufs=2, space="PSUM") as psum,
        ):
            MM = cpool.tile([120, 128], F32)
            WT = cpool.tile([104, 6144], F32)

            w = lambda i: WT[:, i * 512:(i + 1) * 512]
            wd0, wd1, wd2, bd = (w(i) for i in range(4))
            vl0, vl1, vl2, blon = (w(i) for i in range(4, 8))
            ul0, ul1, ul2, blat = (w(i) for i in range(8, 12))

            state = {}

            def qeng(g):
                # loads for g0/g2 + their pair's half-b stores on sync q;
                # g1/g3 + half-a stores on scalar q
                return nc.sync if (g % 2 == 0) else nc.scalar

            def load_half(g, c):
                # one [120, 4096] free-half: 120 x 16KB contiguous runs
                off = (g % G) * B_GRP * BSTRIDE + c * HALF
                X = inp.tile([120, HALF], F32, tag=f"x{c}")
                qeng(g).dma_start(
                    X[:],
                    bass.AP(x, off, [[BSTRIDE, 8], [SLICE, 15], [1, HALF]]))
                state[(g, c)] = X

            def load_consts_head():
                # consts ride q10 first (so group-0's q1 load gets the
                # larger HBM share and pools can start ASAP); MM on q1
                nc.sync.dma_start(MM[:], mmc[:])
                nc.scalar.dma_start(WT[64:104, :], wtc[:])

            def load_consts_tail():
                nc.sync.dma_start(WT[0:40, :], wtc[:])

            def pool_half(g, c):
                # h,w avg-pool (sum) for free-half c -> P2[:, c*256:(c+1)*256]
                X = state.pop((g, c))
                if c == 0:
                    state[("P2", g)] = p2p.tile([120, 512], F32, tag="p2",
                                                name="P2")
                P2 = state[("P2", g)]
                nc.vector.tensor_reduce(
                    P2[:, c * 256:(c + 1) * 256]
                        .rearrange("p (ho wo) -> p ho wo", ho=8),
                    X[:].rearrange("p (ho hs wo ws) -> p ho wo hs ws",
                                   ho=8, hs=4, wo=32, ws=4),
                    mybir.AxisListType.XY, ADD)

            def mm_half(g, c):
                # depth pool (/48) + conv taps for free-half c
                k, half = divmod(g, 2)
                if half == 0 and c == 0:
                    Sdn = psum.tile([104, 512], F32)
                    S0 = psum.tile([104, 512], F32)
                    Sup = psum.tile([104, 512], F32)
                    state[("S", k)] = (Sdn, S0, Sup)
                Sdn, S0, Sup = state[("S", k)]
                P2 = state[("P2", g)]
                if c == 1:
                    state.pop(("P2", g))
                sl = slice(64 * half, 64 * half + 40)
                lo, hi = c * 256, (c + 1) * 256
                nc.tensor.matmul(Sdn[sl, lo:hi], MM[:, 0:40], P2[:, lo:hi],
                                 start=True, stop=True)
                nc.tensor.matmul(S0[sl, lo:hi], MM[:, 40:80], P2[:, lo:hi],
                                 start=True, stop=True)
                nc.tensor.matmul(Sup[sl, lo:hi], MM[:, 80:120], P2[:, lo:hi],
                                 start=True, stop=True)

            def conv_store_pair(k):
                ga, gb = 2 * k, 2 * k + 1
                Sdn, S0, Sup = state.pop(("S", k))
                # depth conv: 3 independent mults, then the add chain
                m = work.tile([104, 512], F32)
                m2 = work.tile([104, 512], F32)
                m3 = work.tile([104, 512], F32)
                nc.vector.tensor_tensor(m[:], wd0, Sdn[:], MULT)
                nc.vector.tensor_tensor(m2[:], wd1, S0[:], MULT)
                nc.vector.tensor_tensor(m3[:], wd2, Sup[:], MULT)
                nc.vector.tensor_tensor(m3[:], m3[:], bd, ADD)
                nc.vector.tensor_tensor(m[:], m[:], m2[:], ADD)
                nc.vector.tensor_tensor(m[:], m[:], m3[:], ADD)
                # relu into lon-padded tile Ydp[p, ho*34 + (wo+1)]
                Ydp = work.tile([104, 544], F32)
                Ydpv = Ydp[:].rearrange("p (ho wp) -> p ho wp", ho=16, wp=34)
                nc.gpsimd.memset(Ydpv[:, :, 0], 0)
                nc.gpsimd.memset(Ydpv[:, :, 33], 0)
                nc.vector.tensor_scalar_max(
                    Ydpv[:, :, 1:33],
                    m[:].rearrange("p (ho wo) -> p ho wo", ho=16), 0.0)

                # lon conv (along wo, free axis)
                m3v = m3[:].rearrange("p (ho wo) -> p ho wo", ho=16)
                mv = m[:].rearrange("p (ho wo) -> p ho wo", ho=16)
                m2v = m2[:].rearrange("p (ho wo) -> p ho wo", ho=16)
                w3 = lambda t: t.rearrange("p (ho wo) -> p ho wo", ho=16)
                nc.vector.tensor_tensor(mv, w3(vl0), Ydpv[:, :, 0:32], MULT)
                nc.vector.tensor_tensor(m2v, w3(vl1), Ydpv[:, :, 1:33], MULT)
                nc.vector.tensor_tensor(m3v, w3(vl2), Ydpv[:, :, 2:34], MULT)
                nc.vector.tensor_tensor(m3v, m3v, w3(blon), ADD)
                nc.vector.tensor_tensor(mv, mv, m2v, ADD)
                nc.vector.tensor_tensor(mv, mv, m3v, ADD)
                # relu into lat-padded tile Ylp[p, (ho+1)*32 + wo]
                Ylp = work.tile([104, 576], F32)
                nc.gpsimd.memset(Ylp[:, 0:32], 0)
                nc.gpsimd.memset(Ylp[:, 544:576], 0)
                nc.vector.tensor_scalar_max(Ylp[:, 32:544], m[:], 0.0)

                # lat conv (along ho, free axis; contiguous slices)
                nc.vector.tensor_tensor(m[:], ul0, Ylp[:, 0:512], MULT)
                nc.vector.tensor_tensor(m2[:], ul1, Ylp[:, 32:544], MULT)
                nc.vector.tensor_tensor(m3[:], ul2, Ylp[:, 64:576], MULT)
                nc.vector.tensor_tensor(m3[:], m3[:], blat, ADD)
                nc.vector.tensor_tensor(m[:], m[:], m2[:], ADD)
                nc.vector.tensor_tensor(m[:], m[:], m3[:], ADD)

                # upsample: relu + h-expand then w-expand, all DVE
                # (gpsimd broadcast APs run ~25x slower — measured — and
                # concurrent gpsimd+DVE SBUF traffic degrades both)
                A = work.tile([104, 2048], F32)  # (ho, hs, wo)
                mv = m[:].rearrange("p (ho wo) -> p ho wo", ho=16)
                for c in range(2):
                    Av = A[:, c * 1024:(c + 1) * 1024].rearrange(
                        "p (ho hs wo) -> p ho hs wo", ho=8, hs=4)
                    mb = mv[:, c * 8:(c + 1) * 8].unsqueeze(2) \
                        .broadcast_to([104, 8, 4, 32])
                    nc.vector.tensor_scalar_max(Av, mb, 0.0)
                for c in range(2):
                    U = outp.tile([104, HALF], F32, tag=f"u{c}")
                    Uw = U[:].rearrange("p (h wo ws) -> p h wo ws", h=32, ws=4)
                    Ab = A[:, c * 1024:(c + 1) * 1024].rearrange(
                        "p (h wo) -> p h wo", h=32) \
                        .unsqueeze(3).broadcast_to([104, 32, 32, 4])
                    nc.vector.tensor_scalar_add(Uw, Ab, 0.0)
                    # stores: half-a (0:40) on scalar q, half-b (64:104)
                    # on sync q; x3 depth replication = 3 DMAs per half
                    for half, g in enumerate((ga, gb)):
                        off = (g % G) * B_GRP * BSTRIDE + c * HALF
                        for di in range(3):
                            eng2 = nc.scalar if half == 0 else nc.sync
                            eng2.dma_start(
                                bass.AP(y, off + di * SLICE,
                                        [[BSTRIDE, 8], [3 * SLICE, 5],
                                         [1, HALF]]),
                                U[64 * half:64 * half + 40, :])

            # software-pipelined emission; emission order = priority order
            for r in range(reps):
                b = r * G
                if r == 0:
                    load_consts_head()
                load_half(b + 0, 0)
                load_half(b + 1, 0)
                load_half(b + 0, 1)
                load_half(b + 1, 1)
                if r == 0:
                    load_consts_tail()
                pool_half(b + 0, 0)
                mm_half(b + 0, 0)
                load_half(b + 2, 0)
                load_half(b + 3, 0)
                pool_half(b + 1, 0)
                mm_half(b + 1, 0)
                load_half(b + 2, 1)
                load_half(b + 3, 1)
                pool_half(b + 0, 1)
                mm_half(b + 0, 1)
                pool_half(b + 1, 1)
                mm_half(b + 1, 1)
                conv_store_pair(b // 2 + 0)
                pool_half(b + 2, 0)
                mm_half(b + 2, 0)
                pool_half(b + 3, 0)
                mm_half(b + 3, 0)
                pool_half(b + 2, 1)
                mm_half(b + 2, 1)
                pool_half(b + 3, 1)
                mm_half(b + 3, 1)
                conv_store_pair(b // 2 + 1)

    nc.compile()
    return nc


_NC_CACHE = {}


def _get_nc(reps: int = 1):
    if reps not in _NC_CACHE:
        _NC_CACHE[reps] = build_nc(reps)
    return _NC_CACHE[reps]


def kernel(x, w_depth, b_depth, w_lon, b_lon, w_lat, b_lat, reps: int = 1,
           **run_kwargs):
    mm, wts = _pack_consts(w_depth, b_depth, w_lon, b_lon, w_lat, b_lat)
    xf = np.ascontiguousarray(np.asarray(x), dtype=np.float32).reshape(N_CORES, CORE_ELEMS)
    in_maps = [{"x": xf[c], "mm": mm, "wts": wts} for c in range(N_CORES)]
    nc = _get_nc(reps)
    res = run_bass_kernel_spmd(nc, in_maps, core_ids=list(range(N_CORES)), **run_kwargs)
    out = np.stack([r["y"] for r in res.results], axis=0)
    out = out.reshape(B, 15, 64, 128, 1)
    if run_kwargs:
        kernel.last_results = res
    return out
